# revision 12
# baseline (speedup 1.0000x reference)
"""Trainium2 Bass kernel for nn_CausalSelfAttention_73744588473038.

GQA causal attention (B=1, T=2048, C=1024, 16 q-heads, 4 kv-heads, hs=64)
with RoPE, per-tensor fake-quant on q/k/att/v/y, FIRE relative-position
bias, and output projection — sharded over 8 NeuronCores (2 q-heads +
their kv head per core; output projection row-parallel via AllToAll).

Key transformations vs the reference:
  * fire_b1/fire_b2 are zero and nd >= 0, so the FIRE MLP collapses to
    bias[h,t1,t2] = coef_h * pninv[t1] * rd[t1-t2]  (coef_h = relu(w1)@w2)
    — a per-partition scalar times a Toeplitz table fed from the host.
  * fake-quant produces 8-bit integer grids; centered integers in
    [-255,255] are exact in bf16, so the attention matmuls run as exact
    integer arithmetic at full bf16 PE rate.
  * softmax output always has min 0 (above-diagonal zeros) / max 1
    (row 0), so the att fake-quant scale is the constant 1/255 + 1e-12.
  * fp32 projections use 3-term bf16 hi/lo split matmuls (error ~2^-18).
  * round() is (x + 1.5*2^23) - 1.5*2^23  (round-half-even, = jnp.round).
  * q/k/v scale min/max: one 8-float AllReduce(max); y min/max rides in
    the AllToAll payload. A dummy AllReduce issued at kernel start hides
    the ~80us first-collective initialization behind the projections.
"""

import os
import sys

for _p in ("/opt/trn_rl_repo", "/root/.axon_site/_ro/trn_rl_repo"):
    if os.path.isdir(_p) and _p not in sys.path:
        sys.path.append(_p)

import numpy as np
import ml_dtypes

N_CORES = 8
T = 2048
C = 1024
N_HEAD = 16
N_KV = 4
HS = C // N_HEAD          # 64
EPS = 1e-6
MAGIC = float(np.float32(1.5 * 2 ** 23))
ROWS_PER_CORE = T // N_CORES          # 256
NBLK = T // 128                       # 16 row-blocks
MW = 2816                             # Toeplitz table width
MOFF = 1920                           # table offset: rel = p + MOFF - q
A2A_BLK = 128 * 256 + 2               # y slice + (negmin, max)

_BUILT = None  # compiled Bass module cache — build once per process


def _bf16(x):
    return np.asarray(x, np.float32).astype(ml_dtypes.bfloat16)


def _hilo(x):
    hi = _bf16(x)
    lo = _bf16(np.asarray(x, np.float32) - hi.astype(np.float32))
    return hi, lo


def _rope_tables():
    inv_freq = (1.0 / (10000.0 ** (np.arange(0, HS, 2, dtype=np.float32) / HS))).astype(np.float32)
    freqs = np.arange(T, dtype=np.float32)[:, None] * inv_freq[None, :]   # (T, 32)
    cos = np.cos(freqs, dtype=np.float32)
    sin = np.sin(freqs, dtype=np.float32)
    cos_full = np.concatenate([cos, cos], axis=1)   # (T, 64)
    sin_full = np.concatenate([sin, sin], axis=1)
    d_idx = np.arange(128) % HS
    cos_t = cos_full[:, d_idx].T.copy()                       # [128, T]
    sin_eff = sin_full[:, (d_idx + 32) % HS].T.copy()         # [128, T]
    return np.ascontiguousarray(cos_t), np.ascontiguousarray(sin_eff)


def _perm_matrix():
    # rope(q) = q*cos + (P @ (q*sin_eff));  P[d, d+32] = -1, P[d+32, d] = +1
    P = np.zeros((128, 128), np.float32)
    for h in range(2):
        b = 64 * h
        for d in range(32):
            P[b + d, b + d + 32] = -1.0
            P[b + 32 + d, b + d] = 1.0
    return np.ascontiguousarray(P.T)  # lhsT for matmul (out = lhsT.T @ rhs)


def host_precompute(x, Wq, Wk, Wv, Wproj, fire_w1, fire_b1, fire_w2, fire_b2,
                    fire_c, fire_L):
    """Build the 8 per-core input maps."""
    xT = np.ascontiguousarray(np.asarray(x, np.float32)[0].T)       # (C, T)
    xt_hi, xt_lo = _hilo(xT)

    c_val = np.float32(np.asarray(fire_c).reshape(-1)[0])
    L_val = np.float32(np.asarray(fire_L).reshape(-1)[0])
    w1 = np.asarray(fire_w1, np.float32)
    w2 = np.asarray(fire_w2, np.float32)
    coef = (np.maximum(w1[0], 0.0).astype(np.float64) @ w2.astype(np.float64)).astype(np.float32)

    pos = np.arange(T, dtype=np.float32)
    pn = np.log(np.abs(c_val * np.maximum(pos, abs(L_val))) + np.float32(1.0),
                dtype=np.float32) + np.float32(EPS)                  # (T,)
    pninv = 1.0 / pn.astype(np.float64)                              # (T,) f64
    pninv_min = float(pninv.min())

    rel = np.arange(-768, 2048, dtype=np.float32)
    rd = np.log(np.abs(c_val * rel) + np.float32(1.0), dtype=np.float32)

    cos_t, sin_eff = _rope_tables()
    permT = _perm_matrix()
    id128_bf = np.eye(128, dtype=ml_dtypes.bfloat16)
    iddup = np.zeros((128, 64), ml_dtypes.bfloat16)
    iddup[0:64] = np.eye(64, dtype=ml_dtypes.bfloat16)
    iddup[64:128] = np.eye(64, dtype=ml_dtypes.bfloat16)
    id128_f32 = np.eye(128, dtype=np.float32)

    wp_hi, wp_lo = _hilo(np.asarray(Wproj, np.float32))
    wp_hi = np.ascontiguousarray(wp_hi.reshape(8, 128, 1024))
    wp_lo = np.ascontiguousarray(wp_lo.reshape(8, 128, 1024))

    q_idx = np.arange(MW)
    p_idx = np.arange(128)[:, None]
    rel_idx = p_idx + MOFF - q_idx[None, :]                          # [128, MW]
    rd_tab = rd[rel_idx + 768]

    in_maps = []
    for core in range(N_CORES):
        h0 = 2 * core
        g = h0 // (N_HEAD // N_KV)
        wqkv = np.concatenate([
            np.asarray(Wq, np.float32)[:, 128 * core:128 * core + 128],
            np.asarray(Wk, np.float32)[:, 64 * g:64 * g + 64],
            np.asarray(Wv, np.float32)[:, 64 * g:64 * g + 64],
        ], axis=1)                                                   # (1024, 256)
        wqkv_hi, wqkv_lo = _hilo(wqkv)

        m_tabs = []
        pncoef = np.zeros((128, 32), np.float32)
        for hl in range(2):
            h = h0 + hl
            ch = float(coef[h])
            mask_val = np.float32(-1e9 / (ch * pninv_min))
            tab = rd_tab.copy()
            tab[rel_idx < 0] = mask_val
            m_tabs.append(np.ascontiguousarray(tab.astype(np.float32)))
            pc = (ch * pninv).astype(np.float32)                     # (T,)
            pncoef[:, 16 * hl:16 * hl + 16] = pc.reshape(NBLK, 128).T

        in_maps.append({
            "xt_hi": xt_hi, "xt_lo": xt_lo,
            "wqkv_hi": np.ascontiguousarray(wqkv_hi.reshape(8, 128, 256)),
            "wqkv_lo": np.ascontiguousarray(wqkv_lo.reshape(8, 128, 256)),
            "wp_hi": wp_hi, "wp_lo": wp_lo,
            "cos_t": cos_t, "sin_eff": sin_eff, "permt": permT,
            "m0": m_tabs[0], "m1": m_tabs[1], "pncoef": pncoef,
            "id128_bf": id128_bf, "iddup": iddup, "id128_f32": id128_f32,
        })
    return in_maps


def build_kernel():
    import concourse.mybir as mybir
    import concourse.tile as tile
    from concourse import bacc

    dt = mybir.dt
    ALU = mybir.AluOpType
    ACTF = mybir.ActivationFunctionType
    AX = mybir.AxisListType

    nc = bacc.Bacc("TRN2", target_bir_lowering=False, debug=False,
                   num_devices=N_CORES)

    f32, bf = dt.float32, dt.bfloat16
    ins = {}
    for name, shape, d in [
        ("xt_hi", (C, T), bf), ("xt_lo", (C, T), bf),
        ("wqkv_hi", (8, 128, 256), bf), ("wqkv_lo", (8, 128, 256), bf),
        ("wp_hi", (8, 128, 1024), bf), ("wp_lo", (8, 128, 1024), bf),
        ("cos_t", (128, T), f32), ("sin_eff", (128, T), f32),
        ("permt", (128, 128), f32),
        ("m0", (128, MW), f32), ("m1", (128, MW), f32),
        ("pncoef", (128, 32), f32),
        ("id128_bf", (128, 128), bf), ("iddup", (128, 64), bf),
        ("id128_f32", (128, 128), f32),
    ]:
        ins[name] = nc.dram_tensor(name, shape, d, kind="ExternalInput")
    out_ext = nc.dram_tensor("out", (ROWS_PER_CORE, C), f32, kind="ExternalOutput")

    RG = [list(range(N_CORES))]
    s_p_const = float(np.float32(np.float32(1.0) / np.float32(255.0) + np.float32(1e-12)))
    inv_sp = float(np.float32(1.0 / np.float64(s_p_const)))

    with tile.TileContext(nc) as tc:
        with (
            tc.tile_pool(name="dram", bufs=8, space="DRAM") as dram,
            tc.tile_pool(name="const", bufs=1) as cpool,
            tc.tile_pool(name="work", bufs=1) as wpool,
            tc.tile_pool(name="stat", bufs=1) as stpool,
        ):
            # ---- dummy AllReduce: warm the collectives engine early ----
            cc_dummy_in = dram.tile([1, 8], f32)
            cc_dummy_out = dram.tile([1, 8], f32)
            nc.gpsimd.dma_start(cc_dummy_in[:], ins["pncoef"][0:1, 0:8])
            nc.gpsimd.collective_compute(
                "AllReduce", ALU.max, replica_groups=RG,
                ins=[cc_dummy_in[:].opt()], outs=[cc_dummy_out[:].opt()])

            # ---- resident constants ----
            cos_t = cpool.tile([128, T], f32)
            sin_eff = cpool.tile([128, T], f32)
            permt = cpool.tile([128, 128], f32)
            pncoef = cpool.tile([128, 32], f32)
            id128_bf = cpool.tile([128, 128], bf)
            iddup = cpool.tile([128, 64], bf)
            id128_f32 = cpool.tile([128, 128], f32)
            nc.sync.dma_start(cos_t[:], ins["cos_t"][:])
            nc.sync.dma_start(sin_eff[:], ins["sin_eff"][:])
            nc.sync.dma_start(permt[:], ins["permt"][:])
            nc.sync.dma_start(pncoef[:], ins["pncoef"][:])
            nc.sync.dma_start(id128_bf[:], ins["id128_bf"][:])
            nc.sync.dma_start(iddup[:], ins["iddup"][:])
            nc.sync.dma_start(id128_f32[:], ins["id128_f32"][:])
            m_tabs = []
            for mname in ("m0", "m1"):
                mt = cpool.tile([128, MW], f32, tag=mname)
                nc.sync.dma_start(mt[:], ins[mname][:])
                m_tabs.append(mt)

            # ---- SBUF tensors that cross phase boundaries ----
            qro = wpool.tile([128, T], f32, tag="qro")
            kvro = wpool.tile([128, T], f32, tag="kvro")  # 0:64 k(rope), 64:128 v
            qc = wpool.tile([128, T], bf, tag="qc")
            kvc = wpool.tile([128, T], bf, tag="kvc")
            kc2 = wpool.tile([128, T], bf, tag="kc2")
            vnat = wpool.tile([128, NBLK, HS], bf, tag="vnat")
            y_sb = wpool.tile([128, T], f32, tag="y_sb")

            # ================= phase 1+2: projections + rope =================
            with tc.tile_pool(name="qt_sb", bufs=1) as qtpool:
                qt = qtpool.tile([128, T], f32, tag="qt")
                kvt = qtpool.tile([128, T], f32, tag="kvt")
                with (
                    tc.tile_pool(name="xt", bufs=3) as xpool,
                    tc.tile_pool(name="wq", bufs=1) as wqpool,
                    tc.tile_pool(name="proj_ps", bufs=1, space="PSUM") as proj_ps,
                ):
                    w_hi = wqpool.tile([128, 8, 256], bf)
                    w_lo = wqpool.tile([128, 8, 256], bf)
                    for kc_ in range(8):
                        nc.sync.dma_start(w_hi[:, kc_, :], ins["wqkv_hi"][kc_])
                        nc.sync.dma_start(w_lo[:, kc_, :], ins["wqkv_lo"][kc_])
                    pstiles = [proj_ps.tile([128, 512], f32, tag="projps%d" % _i, name="projps%d" % _i)
                               for _i in range(8)]
                    for kc_ in range(8):
                        xh = xpool.tile([128, T], bf, tag="xh")
                        xl = xpool.tile([128, T], bf, tag="xl")
                        nc.sync.dma_start(xh[:], ins["xt_hi"][128 * kc_:128 * kc_ + 128, :])
                        nc.sync.dma_start(xl[:], ins["xt_lo"][128 * kc_:128 * kc_ + 128, :])
                        for ti in range(2):
                            wh = w_hi[:, kc_, 128 * ti:128 * ti + 128]
                            wl = w_lo[:, kc_, 128 * ti:128 * ti + 128]
                            for tc_ in range(4):
                                ps = pstiles[4 * ti + tc_]
                                xhs = xh[:, 512 * tc_:512 * tc_ + 512]
                                xls = xl[:, 512 * tc_:512 * tc_ + 512]
                                nc.tensor.matmul(ps[:], wh, xhs, start=(kc_ == 0), stop=False)
                                nc.tensor.matmul(ps[:], wh, xls, start=False, stop=False)
                                nc.tensor.matmul(ps[:], wl, xhs, start=False,
                                                 stop=(kc_ == 7))
                    for ti, dst in ((0, qt), (1, kvt)):
                        for tc_ in range(4):
                            nc.scalar.copy(dst[:, 512 * tc_:512 * tc_ + 512],
                                           pstiles[4 * ti + tc_][:])

                # ---- rope (chunked through [128,512] psum tiles) ----
                with tc.tile_pool(name="rope_ps", bufs=2, space="PSUM") as rope_ps:
                    sq = qtpool.tile([128, T], f32, tag="sq")
                    nc.vector.tensor_mul(sq[:], qt[:], sin_eff[:])
                    skv = qtpool.tile([128, T], f32, tag="skv")
                    nc.gpsimd.tensor_tensor(skv[0:64, :], kvt[0:64, :],
                                            sin_eff[0:64, :], ALU.mult)
                    # qro = qt*cos; kvro[k] = kvt[k]*cos; then += perm(s)
                    nc.vector.tensor_mul(qro[:], qt[:], cos_t[:])
                    nc.gpsimd.tensor_tensor(kvro[0:64, :], kvt[0:64, :],
                                            cos_t[0:64, :], ALU.mult)
                    for tc_ in range(4):
                        sl = slice(512 * tc_, 512 * tc_ + 512)
                        pq = rope_ps.tile([128, 512], f32, tag="permq")
                        nc.tensor.matmul(pq[:], permt[:], sq[:, sl], start=True, stop=True)
                        nc.vector.tensor_add(qro[:, sl], qro[:, sl], pq[:])
                        pk = rope_ps.tile([64, 512], f32, tag="permk")
                        nc.tensor.matmul(pk[0:64, :], permt[0:64, 0:64], skv[0:64, sl],
                                         start=True, stop=True)
                        nc.vector.tensor_add(kvro[0:64, sl], kvro[0:64, sl], pk[0:64, :])
                    nc.scalar.copy(kvro[64:128, :], kvt[64:128, :])

            # ================= phase 2b: stats + scales + quant =================
            with tc.tile_pool(name="st_ps", bufs=1, space="PSUM") as stat_ps:
                # cols: 0 qmax, 1 -qmin, 2 kmax, 3 -kmin, 4 vmax, 5 -vmin
                stat = stpool.tile([128, 6], f32, tag="stat")
                nc.vector.memset(stat[:, 2:6], -3e38)
                nc.vector.tensor_reduce(stat[:, 0:1], qro[:], AX.X, ALU.max)
                nc.vector.tensor_reduce(stat[:, 1:2], qro[:], AX.X, ALU.min, negate=True)
                nc.vector.tensor_reduce(stat[0:64, 2:3], kvro[0:64, :], AX.X, ALU.max)
                nc.vector.tensor_reduce(stat[0:64, 3:4], kvro[0:64, :], AX.X, ALU.min, negate=True)
                nc.vector.tensor_reduce(stat[64:128, 4:5], kvro[64:128, :], AX.X, ALU.max)
                nc.vector.tensor_reduce(stat[64:128, 5:6], kvro[64:128, :], AX.X, ALU.min, negate=True)
                st_ps = stat_ps.tile([6, 128], f32)
                nc.tensor.transpose(st_ps[:], stat[:], id128_f32[:])
                st_t = stpool.tile([6, 128], f32, tag="st_t")
                nc.scalar.copy(st_t[:], st_ps[:])
                g6 = stpool.tile([6, 1], f32, tag="g6")
                nc.vector.tensor_reduce(g6[:], st_t[:], AX.X, ALU.max)

                ar_in = dram.tile([8], f32)
                ar_out = dram.tile([8], f32)
                nc.gpsimd.dma_start(ar_in[0:6], g6[:, 0])
                nc.gpsimd.dma_start(ar_in[6:8], g6[0:2, 0])  # pad
                nc.gpsimd.collective_compute(
                    "AllReduce", ALU.max, replica_groups=RG,
                    ins=[ar_in[:].opt()], outs=[ar_out[:].opt()])
                st8_row = stpool.tile([1, 8], f32, tag="st8r")
                nc.sync.dma_start(st8_row[:], ar_out[:].rearrange("(a b) -> a b", a=1))
                st8 = stpool.tile([128, 8], f32, tag="st8")
                nc.gpsimd.partition_broadcast(st8[:], st8_row[0:1, :])

            # scales: cols 0 s_q, 1 inv_s_q, 2 s_k, 3 inv_s_k, 4 s_v, 5 inv_s_v,
            #         6 alpha, 7 inv_alpha, 8 s_p*s_v
            sc = stpool.tile([128, 12], f32, tag="scales")
            nc.vector.tensor_tensor(sc[:, 0:1], st8[:, 0:1], st8[:, 1:2], ALU.add)
            nc.vector.tensor_scalar(sc[:, 0:1], sc[:, 0:1], float(np.float32(1.0) / np.float32(255.0)), 1e-12, ALU.mult, ALU.add)
            nc.vector.tensor_tensor(sc[:, 2:3], st8[:, 2:3], st8[:, 3:4], ALU.add)
            nc.vector.tensor_scalar(sc[:, 2:3], sc[:, 2:3], float(np.float32(1.0) / np.float32(255.0)), 1e-12, ALU.mult, ALU.add)
            nc.vector.tensor_tensor(sc[:, 4:5], st8[:, 4:5], st8[:, 5:6], ALU.add)
            nc.vector.tensor_scalar(sc[:, 4:5], sc[:, 4:5], float(np.float32(1.0) / np.float32(255.0)), 1e-12, ALU.mult, ALU.add)
            nc.vector.reciprocal(sc[:, 1:2], sc[:, 0:1])
            nc.vector.reciprocal(sc[:, 3:4], sc[:, 2:3])
            nc.vector.reciprocal(sc[:, 5:6], sc[:, 4:5])
            nc.vector.tensor_tensor(sc[:, 6:7], sc[:, 0:1], sc[:, 2:3], ALU.mult)
            nc.vector.tensor_scalar(sc[:, 6:7], sc[:, 6:7], 0.125, None, ALU.mult)
            nc.vector.reciprocal(sc[:, 7:8], sc[:, 6:7])
            nc.vector.tensor_scalar(sc[:, 8:9], sc[:, 4:5], s_p_const, None, ALU.mult)

            betas = stpool.tile([128, 32], f32, tag="betas")
            nc.vector.tensor_scalar(betas[:], pncoef[:], sc[:, 7:8], None, ALU.mult)
            magic_col = stpool.tile([128, 1], f32, tag="magic")
            nc.vector.memset(magic_col[:], MAGIC)

            tmp = stpool.tile([128, T], f32, tag="tmpq")
            nc.vector.tensor_scalar(tmp[:], qro[:], sc[:, 1:2], MAGIC, ALU.mult, ALU.add)
            nc.vector.tensor_scalar(qc[:], tmp[:], MAGIC, None, ALU.subtract)
            kvinv = stpool.tile([128, 1], f32, tag="kvinv")
            nc.vector.tensor_copy(kvinv[0:64, :], sc[0:64, 3:4])
            nc.vector.tensor_copy(kvinv[64:128, :], sc[64:128, 5:6])
            nc.vector.tensor_scalar(tmp[:], kvro[:], kvinv[:], MAGIC, ALU.mult, ALU.add)
            nc.vector.tensor_scalar(kvc[:], tmp[:], MAGIC, None, ALU.subtract)
            nc.sync.dma_start(kc2[0:64, :], kvc[0:64, :])
            nc.sync.dma_start(kc2[64:128, :], kvc[0:64, :])

            with tc.tile_pool(name="vtr_ps", bufs=2, space="PSUM") as vtr_ps:
                for j in range(NBLK):
                    vt_ps = vtr_ps.tile([128, HS], bf)
                    nc.tensor.transpose(vt_ps[:], kvc[64:128, 128 * j:128 * j + 128],
                                        iddup[64:128, :])
                    nc.scalar.copy(vnat[:, j, :], vt_ps[:])

            # ================= phase 3: attention =================
            with (
                tc.tile_pool(name="qk_ps", bufs=4, space="PSUM") as qk_ps,
                tc.tile_pool(name="tr_ps", bufs=2, space="PSUM") as tr_ps,
                tc.tile_pool(name="pv_ps", bufs=1, space="PSUM") as pv_ps,
                tc.tile_pool(name="soft", bufs=2) as soft,
                tc.tile_pool(name="pts", bufs=6) as pts,
                tc.tile_pool(name="pints", bufs=4) as pints,
            ):
                pint_tiles = {}
                pv_tiles = {}

                def emit_qk_softmax(i):
                    L = 128 * (i + 1)
                    nch = (L + 511) // 512
                    for h in range(2):
                        lhs = qc[64 * h:64 * h + 64, 128 * i:128 * i + 128]
                        w_sb = soft.tile([128, 2048], f32, tag="w%d" % h)
                        for c_ in range(nch):
                            wdt = min(512, L - 512 * c_)
                            ps = qk_ps.tile([128, 512], f32)
                            nc.tensor.matmul(
                                ps[:, 0:wdt], lhs,
                                kc2[64 * h:64 * h + 64, 512 * c_:512 * c_ + wdt],
                                start=True, stop=True,
                                tile_position=(64 * h, 0))
                            off = MOFF - 128 * i + 512 * c_
                            nc.vector.scalar_tensor_tensor(
                                w_sb[:, 512 * c_:512 * c_ + wdt],
                                m_tabs[h][:, off:off + wdt],
                                betas[:, 16 * h + i:16 * h + i + 1],
                                ps[:, 0:wdt], ALU.mult, ALU.add)
                        rs = soft.tile([128, 1], f32, tag="rs%d" % h)
                        nc.scalar.activation(w_sb[:, 0:L], w_sb[:, 0:L], ACTF.Exp,
                                             scale=sc[:, 6:7], accum_out=rs[:])
                        rr = soft.tile([128, 1], f32, tag="rr%d" % h)
                        nc.vector.reciprocal(rr[:], rs[:])
                        nc.vector.tensor_scalar(rr[:], rr[:], inv_sp, None, ALU.mult)
                        nc.scalar.activation(w_sb[:, 0:L], w_sb[:, 0:L], ACTF.Identity,
                                             scale=rr[:], bias=magic_col[:, 0:1])
                        pint = pints.tile([128, 2048], bf, tag="pint")
                        nc.vector.tensor_scalar(pint[:, 0:L], w_sb[:, 0:L], MAGIC,
                                                None, ALU.subtract)
                        pint_tiles[(i, h)] = pint

                def emit_tr_pv(i):
                    pvs = []
                    for h in range(2):
                        pv = pv_ps.tile([128, 128], f32, tag="pv%d" % h)
                        pvs.append(pv)
                        pint = pint_tiles.pop((i, h))
                        for j in range(i + 1):
                            tp = tr_ps.tile([128, 128], bf)
                            nc.tensor.transpose(tp[:], pint[:, 128 * j:128 * j + 128],
                                                id128_bf[:])
                            pt = pts.tile([128, 128], bf, tag="pt")
                            if j % 2 == 0:
                                nc.vector.tensor_copy(pt[:], tp[:])
                            else:
                                nc.scalar.copy(pt[:], tp[:])
                            nc.tensor.matmul(
                                pv[64 * h:64 * h + 64, :], vnat[:, j, :], pt[:],
                                start=(j == 0), stop=(j == i),
                                tile_position=(0, 64 * h))
                    nc.scalar.copy(y_sb[0:64, 128 * i:128 * i + 128], pvs[0][0:64, :])
                    nc.scalar.copy(y_sb[64:128, 128 * i:128 * i + 128], pvs[1][64:128, :])

                for i in range(NBLK):
                    emit_qk_softmax(i)
                    if i >= 1:
                        emit_tr_pv(i - 1)
                emit_tr_pv(NBLK - 1)

            # ================= phase 4: y stats + AllToAll =================
            with tc.tile_pool(name="yst_ps", bufs=1, space="PSUM") as yst_psp:
                ystat = stpool.tile([128, 2], f32, tag="ystat")
                nc.vector.tensor_reduce(ystat[:, 0:1], y_sb[:], AX.X, ALU.min, negate=True)
                nc.vector.tensor_reduce(ystat[:, 1:2], y_sb[:], AX.X, ALU.max)
                yst_ps = yst_psp.tile([2, 128], f32)
                nc.tensor.transpose(yst_ps[:], ystat[:], id128_f32[:])
                yst_t = stpool.tile([2, 128], f32, tag="yst_t")
                nc.scalar.copy(yst_t[:], yst_ps[:])
                yg = stpool.tile([2, 1], f32, tag="yg")   # -ymin_raw, ymax_raw
                nc.vector.tensor_reduce(yg[:], yst_t[:], AX.X, ALU.max)

            a2a_in = dram.tile([N_CORES, A2A_BLK], f32)
            a2a_out = dram.tile([N_CORES, A2A_BLK], f32)
            for d in range(N_CORES):
                nc.sync.dma_start(
                    a2a_in[d, 0:128 * 256].rearrange("(p n) -> p n", p=128),
                    y_sb[:, 256 * d:256 * d + 256])
                nc.gpsimd.dma_start(a2a_in[d, 128 * 256:128 * 256 + 2], yg[:, 0])
            nc.gpsimd.collective_compute(
                "AllToAll", ALU.bypass, replica_groups=RG,
                ins=[a2a_in[:].opt()], outs=[a2a_out[:].opt()])

            ycin = wpool.tile([128, N_CORES, 256], f32, tag="ycin")
            mnmx_row = stpool.tile([1, 16], f32, tag="mnmx_row")
            for s in range(N_CORES):
                nc.sync.dma_start(
                    ycin[:, s, :],
                    a2a_out[s, 0:128 * 256].rearrange("(p n) -> p n", p=128))
                nc.gpsimd.dma_start(mnmx_row[0:1, 2 * s:2 * s + 2],
                                    a2a_out[s:s + 1, 128 * 256:128 * 256 + 2])
            mnmx = stpool.tile([128, 16], f32, tag="mnmx")
            nc.gpsimd.partition_broadcast(mnmx[:], mnmx_row[0:1, :])
            ysc = stpool.tile([128, 4], f32, tag="ysc")
            negmn = stpool.tile([128, 1], f32, tag="negmn")
            mxv = stpool.tile([128, 1], f32, tag="mxv")
            mnmx3 = mnmx[:].rearrange("p (a b) -> p a b", b=2)
            nc.vector.tensor_reduce(negmn[:], mnmx3[:, :, 0], AX.X, ALU.max)
            nc.vector.tensor_reduce(mxv[:], mnmx3[:, :, 1], AX.X, ALU.max)
            # ysc: 0 rng_true, 1 s_y, 2 qf = spv/s_y
            nc.vector.tensor_tensor(ysc[:, 0:1], mxv[:], negmn[:], ALU.add)
            nc.vector.tensor_scalar(ysc[:, 0:1], ysc[:, 0:1], sc[:, 8:9], None, ALU.mult)
            nc.vector.tensor_scalar(ysc[:, 1:2], ysc[:, 0:1], float(np.float32(1.0) / np.float32(255.0)), 1e-12, ALU.mult, ALU.add)
            nc.vector.reciprocal(ysc[:, 2:3], ysc[:, 1:2])
            nc.vector.tensor_scalar(ysc[:, 2:3], ysc[:, 2:3], sc[:, 8:9], None, ALU.mult)

            yc = wpool.tile([128, N_CORES * 256], bf, tag="yc")
            ytmp = stpool.tile([128, N_CORES * 256], f32, tag="ytmp")
            ycin_flat = ycin[:].rearrange("p a b -> p (a b)")
            nc.vector.tensor_scalar(ytmp[:], ycin_flat, ysc[:, 2:3], MAGIC, ALU.mult, ALU.add)
            nc.vector.tensor_scalar(yc[:], ytmp[:], MAGIC, None, ALU.subtract)

            # ================= phase 5: output projection =================
            with (
                tc.tile_pool(name="wp", bufs=1) as wppool,
                tc.tile_pool(name="out_ps", bufs=2, space="PSUM") as out_ps,
                tc.tile_pool(name="out_sb", bufs=2) as outpool,
            ):
                wph = wppool.tile([128, 8, 1024], bf)
                wpl = wppool.tile([128, 8, 1024], bf)
                for s in range(8):
                    nc.sync.dma_start(wph[:, s, :], ins["wp_hi"][s])
                    nc.sync.dma_start(wpl[:, s, :], ins["wp_lo"][s])
                yc3 = yc[:].rearrange("p (a b) -> p a b", a=N_CORES)
                for b_ in range(2):
                    for n_ in range(2):
                        ps = out_ps.tile([128, 512], f32)
                        for s in range(8):
                            lhs = yc3[:, s, 128 * b_:128 * b_ + 128]
                            nc.tensor.matmul(ps[:], lhs,
                                             wph[:, s, 512 * n_:512 * n_ + 512],
                                             start=(s == 0), stop=False)
                            nc.tensor.matmul(ps[:], lhs,
                                             wpl[:, s, 512 * n_:512 * n_ + 512],
                                             start=False, stop=(s == 7))
                        osb = outpool.tile([128, 512], f32)
                        nc.scalar.activation(osb[:], ps[:], ACTF.Copy,
                                             scale=ysc[:, 1:2])
                        nc.sync.dma_start(
                            out_ext[128 * b_:128 * b_ + 128, 512 * n_:512 * n_ + 512],
                            osb[:])
    nc.finalize()
    return nc


def _reference_fallback(x, Wq, Wk, Wv, Wproj, fire_w1, fire_b1, fire_w2,
                        fire_b2, fire_c, fire_L):
    """Pure-numpy replication of the reference (used only if structural
    assumptions are violated)."""
    x = np.asarray(x, np.float32)
    B, T_, C_ = x.shape
    H, G = N_HEAD, N_KV
    hs = C_ // H

    def fq(t):
        qmax = np.float32(255.0)
        mn, mx = t.min(), t.max()
        scale = np.float32((mx - mn) / qmax + 1e-12)
        zp = np.round(-mn / scale)
        return ((np.clip(np.round(t / scale) + zp, 0.0, qmax) - zp) * scale).astype(np.float32)

    def rope_np(t):
        D = t.shape[-1]
        inv_freq = 1.0 / (10000.0 ** (np.arange(0, D, 2, dtype=np.float32) / D))
        freqs = np.arange(T_, dtype=np.float32)[:, None] * inv_freq[None, :]
        cos = np.concatenate([np.cos(freqs)] * 2, -1).astype(np.float32)
        sin = np.concatenate([np.sin(freqs)] * 2, -1).astype(np.float32)
        x1, x2 = np.split(t, 2, -1)
        rot = np.concatenate([-x2, x1], -1)
        return t * cos + rot * sin

    q = (x @ Wq).reshape(B, T_, H, hs).transpose(0, 2, 1, 3)
    k = (x @ Wk).reshape(B, T_, G, hs).transpose(0, 2, 1, 3)
    v = (x @ Wv).reshape(B, T_, G, hs).transpose(0, 2, 1, 3)
    q, k = rope_np(q), rope_np(k)
    q, k = fq(q), fq(k)
    kr = np.repeat(k, H // G, 1)
    att = np.einsum('bhqd,bhkd->bhqk', q, kr) / np.sqrt(np.float32(hs))
    pos = np.arange(T_, dtype=np.float32)
    relm = pos[:, None] - pos[None, :]
    thresh = abs(float(np.asarray(fire_L).reshape(-1)[0]))
    cc = np.float32(np.asarray(fire_c).reshape(-1)[0])
    pos_norm = np.maximum(pos, thresh)[:, None]
    rdm = np.log(np.abs(cc * relm) + 1.0, dtype=np.float32)
    pnm = np.log(np.abs(cc * pos_norm) + 1.0, dtype=np.float32) + np.float32(EPS)
    ndm = rdm / pnm
    h1 = np.maximum(ndm[..., None] * np.asarray(fire_w1, np.float32)[0] +
                    np.asarray(fire_b1, np.float32), 0.0)
    bias = h1 @ np.asarray(fire_w2, np.float32) + np.asarray(fire_b2, np.float32)
    att = att + bias.transpose(2, 0, 1)[None]
    causal = np.tril(np.ones((T_, T_), bool))
    att = np.where(causal, att, -np.inf)
    att = att - att.max(-1, keepdims=True)
    att = np.exp(att)
    att = att / att.sum(-1, keepdims=True)
    att = fq(att.astype(np.float32))
    vr = np.repeat(fq(v), H // G, 1)
    y = np.einsum('bhqk,bhkd->bhqd', att, vr).astype(np.float32)
    y = fq(y)
    y = y.transpose(0, 2, 1, 3).reshape(B, T_, C_)
    return (y @ Wproj).astype(np.float32)


def kernel(**inputs):
    x = np.asarray(inputs["x"], np.float32)
    b1 = np.asarray(inputs["fire_b1"], np.float32)
    b2 = np.asarray(inputs["fire_b2"], np.float32)
    w1 = np.asarray(inputs["fire_w1"], np.float32)
    coef = np.maximum(w1[0], 0.0) @ np.asarray(inputs["fire_w2"], np.float32)
    if (x.shape != (1, T, C) or np.any(b1 != 0) or np.any(b2 != 0)
            or np.any(coef == 0.0)):
        return _reference_fallback(**inputs)

    global _BUILT
    if _BUILT is None:
        _BUILT = build_kernel()
    nc = _BUILT

    from concourse.bass_utils import run_bass_kernel_spmd
    in_maps = host_precompute(**inputs)
    res = run_bass_kernel_spmd(nc, in_maps, core_ids=list(range(N_CORES)))
    out = np.concatenate([res.results[c]["out"] for c in range(N_CORES)], axis=0)
    return out[None].astype(np.float32)
